# revision 1
# baseline (speedup 1.0000x reference)
"""Deformable Conv2d (modulated, torchvision-style) Trainium2 Bass kernel.

Data-parallel over batch: 8 samples -> 8 NeuronCores, weights replicated.

Per-core algorithm (all fp32):
  A. offset/mask 3x3 conv as 9 accumulating PE matmuls (channel-major,
     float32r streaming), psum [27,512] chunks.
  B. PE-transpose conv output to pos-major OC[128(w), G(h), 27].
  C. Index math per tap on DVE/ACT: sampling coords, magic-number floor,
     bilinear corner weights folded with sigmoid mask -> M_y0/M_y1
     [128, G, 2]; int16 gather indices in the 16-partition wrapped layout.
  D. Per (superchunk, tap): SWDGE dma_gather of 512B row-pairs (both x
     columns of a tap corner in one descriptor; y0/y1 via base-offset),
     DVE broadcast-multiply by M, accumulating PE transposes back to
     channel-major, then float32r matmul against duplicated conv weights
     accumulating over the 9 taps in PSUM.

The gather source is a host-prepared "pair layout" image: row i of xp is
x_pad[flat i] ++ x_pad[flat i+1] (64+64 channels), zero-padded with P=16
border so all clipped sample coords stay in-bounds and out-of-image
bilinear corners read zeros (exactly the reference's zero padding).
"""

import sys

sys.path.insert(0, "/opt/trn_rl_repo")

import numpy as np

import concourse.bacc as bacc
import concourse.mybir as mybir
from concourse import library_config
from concourse.tile import TileContext
from concourse.bass_utils import run_bass_kernel_spmd

C = 64
H = W = 128
O = 64
P = 16                    # gather pad border
Hp = H + 2 * P
Wp = W + 2 * P
NROW = Hp * Wp
MAGIC = 12582912.0        # 2**23 + 2**22 : fp32 round-to-int magic
N_CORES = 8

_CACHE = {}


def build(G=128, SC=8, reps=1):
    """Build the per-core Bacc module. G = number of h-rows computed
    (128 = full), SC = h-rows per gather superchunk."""
    dt = mybir.dt
    f32, f32r, i16 = dt.float32, dt.float32r, dt.int16
    mult, add, sub = mybir.AluOpType.mult, mybir.AluOpType.add, mybir.AluOpType.subtract
    amax, amin = mybir.AluOpType.max, mybir.AluOpType.min

    assert G % 4 == 0 and G % SC == 0 and (SC * 128) % 512 == 0
    assert (G // 2) % SC == 0
    NQ = SC * 128 // 512          # 512-chunks per superchunk

    nc = bacc.Bacc("TRN2", debug=False)
    xp = nc.dram_tensor("xp", [NROW, 128], f32, kind="ExternalInput")
    xc = nc.dram_tensor("xc", [64, 130 * 130], f32r, kind="ExternalInput")
    wom = nc.dram_tensor("wom", [64, 9 * 27], f32r, kind="ExternalInput")
    w2 = nc.dram_tensor("w2", [128, 9 * 64], f32r, kind="ExternalInput")
    yb = nc.dram_tensor("yb", [128, G], f32, kind="ExternalInput")
    xb = nc.dram_tensor("xb", [128, G], f32, kind="ExternalInput")
    ident = nc.dram_tensor("ident", [128, 128], f32, kind="ExternalInput")
    out = nc.dram_tensor("o", [64, G * 128], f32, kind="ExternalOutput")

    with TileContext(nc, pool_alloc_mode="queue") as tc:
        with (
            tc.tile_pool(name="const", bufs=1) as cpool,
            tc.tile_pool(name="oc", bufs=1) as ocpool,
            tc.tile_pool(name="m", bufs=1) as mpool,
        ):
            # load the gather ucode library up front: the auto-inserted load
            # carries an all-engine fence that would otherwise sit right
            # before the first dma_gather and serialize the whole prefix
            nc.gpsimd.load_library(library_config.mlp)

            wom_sb = cpool.tile([64, 9 * 27], f32r, tag="wom")
            w2_sb = cpool.tile([128, 9 * 64], f32r, tag="w2")
            yb_sb = cpool.tile([128, G], f32, tag="yb")
            xb_sb = cpool.tile([128, G], f32, tag="xb")
            id_sb = cpool.tile([128, 128], f32, tag="ident")
            nc.sync.dma_start(wom_sb[:], wom[:])
            nc.sync.dma_start(w2_sb[:], w2[:])
            nc.sync.dma_start(yb_sb[:], yb[:])
            nc.sync.dma_start(xb_sb[:], xb[:])
            nc.sync.dma_start(id_sb[:], ident[:])

            oc = ocpool.tile([128, G * 27], f32, tag="oc")      # [w, h, ch27]
            m0s = [mpool.tile([128, G * 2], f32, tag=f"m0_{k}", name=f"m0_{k}") for k in range(9)]
            m1s = [mpool.tile([128, G * 2], f32, tag=f"m1_{k}", name=f"m1_{k}") for k in range(9)]
            # wrapped gather indices, one tile per tap (all 128 partitions =
            # 8 replicated Q7-core groups of the [16, n/16] wrapped table)
            idxws = [mpool.tile([128, G * 8], i16, tag=f"ix_{k}", name=f"ix_{k}") for k in range(9)]

            for _ in range(reps):
                # phases A-C are pipelined over g-halves so the first
                # gathers start after only half the conv
                NH = 2
                GH = G // NH
                with (
                    tc.tile_pool(name="xcp", bufs=1) as xcpool,
                    tc.tile_pool(name="ccm", bufs=2) as ccmpool,
                    tc.tile_pool(name="pconv", bufs=2, space="PSUM") as pcv,
                    tc.tile_pool(name="ptr", bufs=2, space="PSUM") as ptr,
                    tc.tile_pool(name="scratch", bufs=8) as scp,
                ):
                    xc_sb = xcpool.tile([64, 130 * 130], f32r, tag="xc")
                    nc.sync.dma_start(xc_sb[:], xc[:])
                    xv = xc_sb[:].rearrange("p (h w) -> p h w", h=130, w=130)
                    ocv = oc[:].rearrange("p (g j) -> p g j", g=G, j=27)
                    msk = mpool.tile([128, G * 9], f32, tag="msk")
                    mskv = msk[:].rearrange("p (g j) -> p g j", g=G, j=9)
                    for hf in range(NH):
                        g0 = hf * GH
                        # ---- conv + transpose for this half ----
                        for pc in range(g0 // 4, (g0 + GH) // 4):
                            ps = pcv.tile([27, 512], f32, tag="pconv")
                            for k in range(9):
                                ki, kj = k // 3, k % 3
                                rhs = xv[:, 4 * pc + ki:4 * pc + ki + 4, kj:kj + 128]
                                nc.tensor.matmul(
                                    ps[:],
                                    lhsT=wom_sb[:, k * 27:(k + 1) * 27],
                                    rhs=rhs,
                                    start=(k == 0),
                                    stop=(k == 8),
                                )
                            cct = ccmpool.tile([27, 512], f32, tag="ccm")
                            nc.scalar.copy(cct[:], ps[:])
                            for gg in range(4):
                                g = 4 * pc + gg
                                pt = ptr.tile([128, 27], f32, tag="ptr")
                                nc.tensor.matmul(
                                    pt[:],
                                    lhsT=cct[:, gg * 128:(gg + 1) * 128],
                                    rhs=id_sb[0:27, 0:27],
                                    is_transpose=True,
                                    start=True,
                                    stop=True,
                                )
                                nc.scalar.copy(oc[:, g * 27:(g + 1) * 27], pt[:])
                        # ---- index math for this half ----
                        hs = slice(g0, g0 + GH)
                        nc.scalar.activation(
                            mskv[:, hs, :], ocv[:, hs, 18:27],
                            mybir.ActivationFunctionType.Sigmoid,
                        )
                        for k in range(9):
                            m0v = m0s[k][:].rearrange("p (g j) -> p g j", g=G, j=2)
                            m1v = m1s[k][:].rearrange("p (g j) -> p g j", g=G, j=2)
                            ki, kj = k // 3, k % 3

                            def axis(base_sb, off_ap, lo, hi):
                                pv = scp.tile([128, GH], f32, tag="pv")
                                pvc = scp.tile([128, GH], f32, tag="pvc")
                                t = scp.tile([128, GH], f32, tag="t")
                                v0f = scp.tile([128, GH], f32, tag="v0f")
                                fv = scp.tile([128, GH], f32, tag="fv")
                                gv = scp.tile([128, GH], f32, tag="gv")
                                CP = mybir.ActivationFunctionType.Copy
                                nc.vector.tensor_tensor(pv[:], off_ap, base_sb[:, hs], add)
                                nc.vector.tensor_scalar(pvc[:], pv[:], float(lo), float(hi), amax, amin)
                                th = scp.tile([128, GH], f32, tag="th")
                                nc.scalar.activation(th[:], pvc[:], CP, bias=-0.5)
                                nc.scalar.activation(t[:], th[:], CP, bias=MAGIC)
                                nc.scalar.activation(v0f[:], t[:], CP, bias=-MAGIC)
                                nc.vector.tensor_tensor(fv[:], pvc[:], v0f[:], sub)
                                nc.scalar.activation(gv[:], fv[:], CP, scale=-1.0, bias=1.0)
                                return v0f, fv, gv

                            y0f, fy, gy = axis(yb_sb, ocv[:, hs, 2 * k], -15.0 - (ki - 1), 142.0 - (ki - 1))
                            x0f, fx, gx = axis(xb_sb, ocv[:, hs, 2 * k + 1], -15.0 - (kj - 1), 142.0 - (kj - 1))

                            my0 = scp.tile([128, GH], f32, tag="my0")
                            my1 = scp.tile([128, GH], f32, tag="my1")
                            nc.vector.tensor_tensor(my0[:], mskv[:, hs, k], gy[:], mult)
                            nc.vector.tensor_tensor(my1[:], mskv[:, hs, k], fy[:], mult)
                            nc.vector.tensor_tensor(m0v[:, hs, 0], my0[:], gx[:], mult)
                            nc.vector.tensor_tensor(m0v[:, hs, 1], my0[:], fx[:], mult)
                            nc.vector.tensor_tensor(m1v[:, hs, 0], my1[:], gx[:], mult)
                            nc.vector.tensor_tensor(m1v[:, hs, 1], my1[:], fx[:], mult)

                            cst = float((ki - 1 + P) * Wp + (kj - 1 + P))
                            idf = scp.tile([128, GH], f32, tag="idf")
                            idf2 = scp.tile([128, GH], f32, tag="idf2")
                            idx16 = scp.tile([128, GH], i16, tag="idx16")
                            nc.scalar.activation(
                                idf[:], y0f[:], mybir.ActivationFunctionType.Copy,
                                scale=float(Wp), bias=cst,
                            )
                            nc.vector.tensor_tensor(idf2[:], idf[:], x0f[:], add)
                            nc.vector.tensor_copy(idx16[:], idf2[:])
                            iwv = idxws[k][:].rearrange("p (g j) -> p g j", g=G, j=8)
                            for pp in range(8):
                                nc.sync.dma_start(
                                    iwv[0:16, hs, pp], idx16[pp * 16:(pp + 1) * 16, :]
                                )
                            # gather ucode: each of the 8 Q7 cores reads its
                            # own 16-partition group -> replicate the table
                            # (doubling tree: 3 DMAs instead of 7)
                            nc.sync.dma_start(iwv[16:32, hs, :], iwv[0:16, hs, :])
                            nc.sync.dma_start(iwv[32:64, hs, :], iwv[0:32, hs, :])
                            nc.sync.dma_start(iwv[64:128, hs, :], iwv[0:64, hs, :])
                        # ---------------- phase D: gather / combine / matmul ----------------
                        with (
                            tc.tile_pool(name="gp", bufs=3) as gp,
                            tc.tile_pool(name="sp", bufs=3) as sp,
                            tc.tile_pool(name="stp", bufs=2) as stp,
                            tc.tile_pool(name="op", bufs=2) as op,
                            tc.tile_pool(name="pst", bufs=2, space="PSUM") as pstp,
                            tc.tile_pool(name="po", bufs=NQ, space="PSUM") as pop,
                        ):
                            for sc in range(hf * (G // SC // NH), (hf + 1) * (G // SC // NH)):
                                pos = [
                                    pop.tile([64, 512], f32, tag="po", name=f"po{q}")
                                    for q in range(NQ)
                                ]
                                for k in range(9):
                                    gy0 = gp.tile([128, SC * 128], f32, tag="g")
                                    gy1 = gp.tile([128, SC * 128], f32, tag="g")
                                    idxs = idxws[k][:, sc * SC * 8:(sc + 1) * SC * 8]
                                    nc.gpsimd.dma_gather(
                                        gy0[:].rearrange("p (s e) -> p s e", s=SC, e=128),
                                        xp[0:NROW - Wp, :],
                                        idxs,
                                        SC * 128,
                                        SC * 128,
                                        128,
                                    )
                                    nc.gpsimd.dma_gather(
                                        gy1[:].rearrange("p (s e) -> p s e", s=SC, e=128),
                                        xp[Wp:NROW, :],
                                        idxs,
                                        SC * 128,
                                        SC * 128,
                                        128,
                                    )
                                    sy0 = sp.tile([128, SC * 128], f32, tag="s")
                                    sy1 = sp.tile([128, SC * 128], f32, tag="s")
                                    for dst, g_t, m_t in ((sy0, gy0, m0s[k]), (sy1, gy1, m1s[k])):
                                        mv = m_t[:].rearrange("p (g j) -> p g j", g=G, j=2)
                                        m_ap = (
                                            mv[:, sc * SC:(sc + 1) * SC, :]
                                            .unsqueeze(3)
                                            .to_broadcast([128, SC, 2, 64])
                                        )
                                        nc.vector.tensor_tensor(
                                            dst[:].rearrange("p (s j c) -> p s j c", s=SC, j=2, c=64),
                                            g_t[:].rearrange("p (s j c) -> p s j c", s=SC, j=2, c=64),
                                            m_ap,
                                            mult,
                                        )
                                    for q in range(NQ):
                                        pst = pstp.tile([128, 512], f32, tag="pst")
                                        for gg in range(4):
                                            gl = q * 4 + gg
                                            dst = pst[:, gg * 128:(gg + 1) * 128]
                                            nc.tensor.matmul(
                                                dst, lhsT=sy0[:, gl * 128:(gl + 1) * 128],
                                                rhs=id_sb[:], is_transpose=True,
                                                start=True, stop=False,
                                            )
                                            nc.tensor.matmul(
                                                dst, lhsT=sy1[:, gl * 128:(gl + 1) * 128],
                                                rhs=id_sb[:], is_transpose=True,
                                                start=False, stop=True,
                                            )
                                        sT = stp.tile([128, 512], f32r, tag="st")
                                        nc.scalar.copy(sT[:], pst[:])
                                        nc.tensor.matmul(
                                            pos[q][:],
                                            lhsT=w2_sb[:, k * 64:(k + 1) * 64],
                                            rhs=sT[:],
                                            start=(k == 0),
                                            stop=(k == 8),
                                        )
                                for q in range(NQ):
                                    ob = op.tile([64, 512], f32, tag="ob")
                                    nc.scalar.copy(ob[:], pos[q][:])
                                    nc.sync.dma_start(
                                        out[:, sc * SC * 128 + q * 512: sc * SC * 128 + (q + 1) * 512],
                                        ob[:],
                                    )

    nc.finalize()
    return nc


def host_prep(x_s, w_conv, w_offset, w_mask, G=128):
    """Per-sample input layouts (pure layout transforms + constants)."""
    x_s = np.ascontiguousarray(x_s, np.float32)
    flat = np.zeros((Hp, Wp, C), np.float32)
    flat[P:P + H, P:P + W, :] = x_s.transpose(1, 2, 0)
    flat = flat.reshape(NROW, C)
    xp = np.zeros((NROW, 128), np.float32)
    xp[:, :64] = flat
    xp[:NROW - 1, 64:] = flat[1:]
    xc = np.zeros((C, 130, 130), np.float32)
    xc[:, 1:1 + H, 1:1 + W] = x_s
    wom = np.zeros((C, 9, 27), np.float32)
    w2 = np.zeros((128, 9, 64), np.float32)
    for ki in range(3):
        for kj in range(3):
            k = ki * 3 + kj
            wom[:, k, 0:18] = w_offset[:, :, ki, kj].T
            wom[:, k, 18:27] = w_mask[:, :, ki, kj].T
            w2[0:64, k, :] = w_conv[:, :, ki, kj].T
            w2[64:128, k, :] = w_conv[:, :, ki, kj].T
    yb = np.broadcast_to(np.arange(G, dtype=np.float32)[None, :], (128, G)).copy()
    xb = np.broadcast_to(np.arange(128, dtype=np.float32)[:, None], (128, G)).copy()
    ident = np.eye(128, dtype=np.float32)
    return {
        "xp": xp,
        "xc": np.ascontiguousarray(xc.reshape(C, 130 * 130)),
        "wom": np.ascontiguousarray(wom.reshape(C, 9 * 27)),
        "w2": np.ascontiguousarray(w2.reshape(128, 9 * 64)),
        "yb": yb,
        "xb": xb,
        "ident": ident,
    }


def _get_nc():
    key = (128, 8, 1)
    if key not in _CACHE:
        _CACHE[key] = build(*key)
    return _CACHE[key]


def kernel(x, w_conv, w_offset, w_mask):
    x = np.ascontiguousarray(x, np.float32)
    nc = _get_nc()
    in_maps = [
        host_prep(x[b], np.asarray(w_conv, np.float32),
                  np.asarray(w_offset, np.float32), np.asarray(w_mask, np.float32))
        for b in range(N_CORES)
    ]
    res = run_bass_kernel_spmd(nc, in_maps, core_ids=list(range(N_CORES)))
    return np.stack([res.results[b]["o"].reshape(O, H, W) for b in range(N_CORES)])

